# revision 18
# baseline (speedup 1.0000x reference)
"""AI4DEM contact-force stencil on 8 TRN2 NeuronCores.

Math: for each neighbor offset o=(oy,ox) in the 5x5 window,
  dx = x - shift(x, o), dy likewise, dist = sqrt(dx^2+dy^2)
  Fx_o = where(dist < 2d, kn*(dist-2d)/max(eps,dist) * dx, 0)
       = -kn * relu(2d/dist - 1) * dx    (clamped at dist<eps)
  fx = mask * sum_o Fx_o

Mirror symmetry: Fx_{-o}(p) = -Fx_o(p+o), so only the 12 half-offsets
H = {oy>0} u {oy=0, ox>0} are computed:
  fx(p) = sum_{o in H} px_o(p) - px_o(p+o),   px_o = w_o * dx_o
The shifted term is accumulated by TensorE matmuls with +/-1 shift
matrices into PSUM (partition shifts), free-dim reads handle ox.
The 2 rows past each 128-row tile (and past the core's 256-row block)
come from one packed 48-partition "seam" chain over host-gathered rows
(each seam row pre-shifted by its own ox so all 5 ox-blocks merge into
a single lhsT per tile => 16 seam matmuls instead of 80).

Engine split per offset-chain (the elementwise bottleneck):
 - x and y slabs are packed side by side in ONE fp16 tensor, so dx and
   dy come from a single paired DVE subtract in 2x mode.
 - squares: one paired ACT Square (ACT is dtype-insensitive).
 - sq add + the contact weight on DVE; the weight uses tensor_scalar
   (max c, sub c) in 4x mode: relu(a*r-1) = a*(max(r,1/a)-1/a), with a
   folded into the final output scale.
 - px multiply on DVE (fp16 2x), py multiply on the otherwise-idle
   GpSimd, output scale+mask stt on GpSimd.

1/dist is one ACT op Abs_reciprocal_sqrt(sq + 4.1e-6); grids are
pre-scaled by 64 then cast to fp16 on the host so fp16 sq stays normal
for all dist >= ~eps; all scale factors fold into existing constants.
"""

import numpy as np

NY = NX = 2048
NCORES = 8
ROWS = NY // NCORES          # 256 rows per core
TILE = 128
NT = ROWS // TILE            # 2 row-tiles per core
W = NX + 4                   # px width (2-col halo)
WS = NX + 8                  # slab width (4-col halo)
EPS = 1e-4
SCALE = 64.0

# half-offsets, oy ascending so oy=0 pairs start right after the first loads
HOFF = [(0, 1), (0, 2)] + [(oy, ox) for oy in (1, 2) for ox in (-2, -1, 0, 1, 2)]
NH = len(HOFF)               # 12

_cache = {}
LAST_RESULTS = None


def _build(d_val: float, kn_val: float):
    import concourse.tile as tile
    from concourse import bacc, mybir

    f32 = mybir.dt.float32
    f16 = mybir.dt.float16
    AF = mybir.ActivationFunctionType
    OP = mybir.AluOpType

    nc = bacc.Bacc("TRN2", target_bir_lowering=False, debug=False,
                   enable_asserts=False, num_devices=NCORES)
    # packed x|y slabs, fp16: [row, half(x=0,y=1), col]
    zs_ext = nc.declare_dram_parameter("zs", [ROWS + 4, 2, WS], f16,
                                       isOutput=False)
    ms_ext = nc.declare_dram_parameter("ms", [ROWS, NX], f16, isOutput=False)
    # lhs blocks: [I, -S0, -S1, -S2, _, I-S1, I-S2] where Sk shifts partitions
    lhs_ext = nc.declare_dram_parameter("lhs", [128, 6 * 128], f16,
                                        isOutput=False)
    # merged boundary lhsT per t: [48, 2*128]
    lhsb_ext = nc.declare_dram_parameter("lhsb", [48, 2 * 128], f16,
                                         isOutput=False)
    # seam a/b rows, packed x|y, per-row pre-shifted by ox
    sa_ext = nc.declare_dram_parameter("sa", [4 * NH, 2, W], f16,
                                       isOutput=False)
    sb_ext = nc.declare_dram_parameter("sb", [4 * NH, 2, W], f16,
                                       isOutput=False)
    out_ext = nc.declare_dram_parameter("out", [2, ROWS, NX], f32,
                                        isOutput=True)

    SP = 4 * NH              # 48 seam partitions
    cw = 1.0 / (2.0 * SCALE * d_val)     # 1/a for the weight tensor_scalar

    with tile.TileContext(nc) as tc:
        with tc.tile_pool(name="const", bufs=1) as cpool, \
             tc.tile_pool(name="zin", bufs=2) as zpool, \
             tc.tile_pool(name="deep", bufs=4) as dpool, \
             tc.tile_pool(name="sqp", bufs=2) as qpool, \
             tc.tile_pool(name="tmp", bufs=2) as tpool, \
             tc.tile_pool(name="pxy", bufs=3) as ppool, \
             tc.tile_pool(name="outp", bufs=2) as opool, \
             tc.tile_pool(name="acc", bufs=1, space="PSUM") as psum_pool:

            lhs_t = cpool.tile([128, 6 * 128], f16)
            nc.sync.dma_start(lhs_t[:], lhs_ext[:])
            lhsb_t = cpool.tile([48, 2 * 128], f16)
            nc.sync.dma_start(lhsb_t[:], lhsb_ext[:])
            floor_b = cpool.tile([128, 1], f32)
            nc.vector.memset(floor_b[:], 4.1e-6)
            negc_b = cpool.tile([128, 1], f32)
            nc.vector.memset(negc_b[:], -cw)
            pxm = cpool.tile([SP, W], f16, tag="pxm")
            pym = cpool.tile([SP, W], f16, tag="pym")

            oscale = -2.0 * float(kn_val) * float(d_val)
            NCH = NT * NH                    # 24 interior chains
            # per-tile processing order: ox==0 offsets last (cheap tail: the
            # merged I-S pass needs half the matmuls of an ox!=0 offset)
            POFF = [0, 1, 2, 3, 5, 6, 7, 8, 10, 11, 4, 9]
            chains = [(t, oi) for t in range(NT) for oi in POFF]
            # w via ACT Relu on these chains (engine balance), else DVE ts
            w_on_act = {g for g, (t, oi) in enumerate(chains)
                        if oi % 2 == 0 and oi != 6}

            Z = {}       # Z[t][s] slab tiles
            MS = {}      # mask per tile
            FX = {}      # psum chunk tiles per tile
            FY = {}
            st = {}      # per-chain tile state
            sst = {}     # seam state

            def emit_zdma(t):
                t0 = t * TILE
                Z[t] = {}
                for s in (0, -1, -2):
                    zt = zpool.tile([TILE, 2, WS], f16, tag=f"zs{s}",
                                    name=f"z{t}{s}")
                    nc.sync.dma_start(
                        zt[:], zs_ext[t0 + s + 2: t0 + s + 2 + TILE, :, :])
                    Z[t][s] = zt
                mt = zpool.tile([TILE, NX], f16, tag="mask", name=f"m{t}")
                nc.sync.dma_start(mt[:], ms_ext[t0: t0 + TILE, :])
                MS[t] = mt

            def emit_sub(g):
                t, oi = chains[g]
                oy, ox = HOFF[oi]
                dxy = dpool.tile([TILE, 2, W], f16, tag="dxy", name=f"dxy{g}")
                nc.vector.tensor_sub(
                    dxy[:], Z[t][0][:, :, 2: 2 + W],
                    Z[t][-oy][:, :, 2 - ox: 2 - ox + W])
                st[g] = {"dxy": dxy}

            def emit_sq(g):
                sq2 = qpool.tile([TILE, 2, W], f16, tag="sq2", name=f"sq2{g}")
                nc.scalar.activation(sq2[:], st[g]["dxy"][:], AF.Square)
                st[g]["sq2"] = sq2

            def emit_add(g):
                sq2 = st[g].pop("sq2")
                rec = tpool.tile([TILE, W], f16, tag="rec", name=f"rec{g}")
                nc.vector.tensor_add(rec[:], sq2[:, 0, :], sq2[:, 1, :])
                st[g]["rec"] = rec

            def emit_rsqrt(g):
                rec = st[g]["rec"]
                nc.scalar.activation(rec[:], rec[:], AF.Abs_reciprocal_sqrt,
                                     bias=floor_b[:TILE])
                if g in w_on_act:
                    w = tpool.tile([TILE, W], f16, tag="w", name=f"w{g}")
                    nc.scalar.activation(w[:], rec[:], AF.Relu,
                                         bias=negc_b[:TILE])
                    st[g]["w"] = w

            def emit_tail(g):
                t, oi = chains[g]
                oy, ox = HOFF[oi]
                first = g % NH == 0
                last = g % NH == NH - 1
                fin = g == NCH - 1      # last chain of the whole kernel
                if g not in w_on_act:
                    w = tpool.tile([TILE, W], f16, tag="w", name=f"w{g}")
                    nc.vector.tensor_scalar(w[:], st[g]["rec"][:], cw, cw,
                                            OP.max, OP.subtract)
                    st[g]["w"] = w
                w = st[g]["w"]
                dxy = st[g]["dxy"]
                px = ppool.tile([TILE, W], f16, tag="px", name=f"px{g}")
                py = ppool.tile([TILE, W], f16, tag="py", name=f"py{g}")
                nc.vector.tensor_mul(px[:], w[:], dxy[:, 0, :])
                nc.vector.tensor_mul(py[:], w[:], dxy[:, 1, :])
                if first:
                    FX[t] = [psum_pool.tile([TILE, 512], f32, tag=f"fx{k}",
                                            name=f"fx{t}{k}")
                             for k in range(NX // 512)]
                    FY[t] = [psum_pool.tile([TILE, 512], f32, tag=f"fy{k}",
                                            name=f"fy{t}{k}")
                             for k in range(NX // 512)]
                for ps, acc, drain_ks in ((px, FX[t], [0, 1, 2, 3]),
                                          (py, FY[t], None)):
                    if ox == 0:
                        for k in range(NX // 512):
                            nc.tensor.matmul(
                                acc[k][:],
                                lhs_t[:, 128 * (3 + oy): 128 * (4 + oy)],
                                ps[:, 2 + 512 * k: 2 + 512 * k + 512],
                                start=first, stop=last)
                            if fin:
                                # drain each chunk right after its stop mm
                                emit_drain_one(t, k, FX[t] if drain_ks
                                               else FY[t],
                                               0 if drain_ks else 1)
                    else:
                        for k in range(NX // 512):
                            nc.tensor.matmul(
                                acc[k][:], lhs_t[:, 0:128],
                                ps[:, 2 + 512 * k: 2 + 512 * k + 512],
                                start=first, stop=False)
                        for k in range(NX // 512):
                            nc.tensor.matmul(
                                acc[k][:],
                                lhs_t[:, 128 * (1 + oy): 128 * (2 + oy)],
                                ps[:, 2 + 512 * k + ox:
                                   2 + 512 * k + ox + 512],
                                start=False, stop=last)
                            if fin:
                                emit_drain_one(t, k, FX[t] if drain_ks
                                               else FY[t],
                                               0 if drain_ks else 1)
                del st[g]

            def emit_seam_mms(t):
                for pm, acc in ((pxm, FX[t]), (pym, FY[t])):
                    for k in range(NX // 512):
                        nc.tensor.matmul(
                            acc[k][:], lhsb_t[:, 128 * t: 128 * t + 128],
                            pm[:, 2 + 512 * k: 2 + 512 * k + 512],
                            start=False, stop=False)

            def emit_drain_one(t, k, acc, half, nsplit=1):
                t0 = t * TILE
                tagc = "fxsb" if half == 0 else "fysb"
                sb_t = opool.tile([TILE, 512], f32, tag=tagc,
                                  name=f"{tagc}{t}{k}")
                nc.vector.scalar_tensor_tensor(
                    sb_t[:], acc[k][:], oscale,
                    MS[t][:, 512 * k: 512 * k + 512], OP.mult, OP.mult)
                sw = 512 // nsplit
                for j in range(nsplit):
                    c0 = 512 * k + sw * j
                    nc.gpsimd.dma_start(
                        out_ext[half, t0: t0 + TILE, c0: c0 + sw],
                        sb_t[:, sw * j: sw * j + sw])

            def emit_drain(t, ks):
                for k in ks:
                    emit_drain_one(t, k, FX[t], 0)
                    emit_drain_one(t, k, FY[t], 1)

            def emit_seam_stage(stage):
                if stage == 0:
                    sst["sa"] = zpool.tile([SP, 2, W], f16, tag="sa", bufs=1, name="sa")
                    nc.sync.dma_start(sst["sa"][:], sa_ext[:])
                    sst["sb"] = zpool.tile([SP, 2, W], f16, tag="sb", bufs=1, name="sb")
                    nc.sync.dma_start(sst["sb"][:], sb_ext[:])
                elif stage == 1:
                    sst["dm"] = cpool.tile([SP, 2, W], f16, tag="sdm", name="sdm")
                    nc.vector.tensor_sub(sst["dm"][:], sst["sa"][:],
                                         sst["sb"][:])
                elif stage == 2:
                    sst["s2m"] = cpool.tile([SP, 2, W], f16, tag="ss2", name="ss2")
                    nc.scalar.activation(sst["s2m"][:], sst["dm"][:],
                                         AF.Square)
                elif stage == 3:
                    s2m = sst["s2m"]
                    sst["recm"] = cpool.tile([SP, W], f16, tag="srec", name="srec")
                    nc.vector.tensor_add(sst["recm"][:], s2m[:, 0, :],
                                         s2m[:, 1, :])
                elif stage == 4:
                    nc.scalar.activation(sst["recm"][:], sst["recm"][:],
                                         AF.Abs_reciprocal_sqrt,
                                         bias=floor_b[:SP])
                elif stage == 5:
                    sst["wm"] = cpool.tile([SP, W], f16, tag="swm", name="swm")
                    nc.scalar.activation(sst["wm"][:], sst["recm"][:],
                                         AF.Relu, bias=negc_b[:SP])
                elif stage == 6:
                    nc.vector.tensor_mul(pxm[:], sst["wm"][:],
                                         sst["dm"][:, 0, :])
                elif stage == 7:
                    nc.vector.tensor_mul(pym[:], sst["wm"][:],
                                         sst["dm"][:, 1, :])

            # ---- stage-skewed software pipeline over the 24 chains
            emit_zdma(0)
            for e in range(NCH + 2):
                if e == 0:
                    emit_sub(0)
                    emit_sub(1)
                    emit_sq(0)
                if e < NCH:
                    emit_add(e)
                if e + 1 < NCH:
                    emit_sq(e + 1)
                if e + 2 < NCH:
                    emit_sub(e + 2)
                if e < NCH:
                    emit_rsqrt(e)
                if 1 <= e < 9:
                    emit_seam_stage(e - 1)
                if e == 6:
                    emit_zdma(1)
                g = e - 1
                if 0 <= g < NCH:
                    emit_tail(g)
                    if g == 8:
                        emit_seam_mms(0)
                    if g == NH + 1:
                        emit_seam_mms(1)
                    if g == NH - 1:
                        emit_drain(0, [0, 1])
                    if g == NH:
                        emit_drain(0, [2, 3])

    nc.compile()
    return nc


def _host_inputs(gx, gy, ms):
    """Per-core input dict list. gx/gy scaled by 64 and cast to fp16."""
    eye = np.eye(128, dtype=np.float16)
    lhs = np.zeros((128, 6 * 128), dtype=np.float16)
    lhs[:, 0:128] = eye
    for oy in (0, 1, 2):
        blk = np.zeros((128, 128), dtype=np.float16)
        for m in range(128 - oy):
            blk[m + oy, m] = -1.0
        lhs[:, 128 * (1 + oy): 128 * (2 + oy)] = blk
        if oy > 0:
            lhs[:, 128 * (3 + oy): 128 * (4 + oy)] = eye + blk
    # merged boundary lhsT: rows are disjoint per offset, so sum over ox
    lhsb = np.zeros((48, 2 * 128), dtype=np.float16)
    for t in (0, 1):
        blk = np.zeros((48, 128), dtype=np.float16)
        for oi, (oy, ox) in enumerate(HOFF):
            if oy == 0:
                continue
            for m in range(128 - oy, 128):
                blk[4 * oi + 2 * t + (m + oy - 128), m] = -1.0
        lhsb[:, 128 * t: 128 * t + 128] = blk

    cols = np.arange(-2, NX + 2) % NX         # width W, col u -> u-2
    colss = np.arange(-4, NX + 4) % NX        # width WS, col v -> v-4
    in_maps = []
    for i in range(NCORES):
        r0 = i * ROWS
        rows = np.arange(r0 - 2, r0 + ROWS + 2) % NY
        # seam rows per (oi, j): j in {0,1}: r0+128+j ; j in {2,3}: r0+256+(j-2)
        # each row pre-shifted by its own ox: sa col c -> u = c-2+ox
        sa_rows = np.empty(4 * NH, dtype=np.int64)
        sb_rows = np.empty(4 * NH, dtype=np.int64)
        sa_cols = np.empty((4 * NH, W), dtype=np.int64)
        for oi, (oy, ox) in enumerate(HOFF):
            for j in range(4):
                row = r0 + 128 + j if j < 2 else r0 + 256 + (j - 2)
                sa_rows[4 * oi + j] = row % NY
                sb_rows[4 * oi + j] = (row - oy) % NY
                sa_cols[4 * oi + j] = (cols + ox) % NX
        sa = np.stack([gx[sa_rows[:, None], sa_cols],
                       gy[sa_rows[:, None], sa_cols]], axis=1)
        sb = np.stack([gx[sb_rows][:, cols],
                       gy[sb_rows][:, cols]], axis=1)
        zs = np.stack([gx[np.ix_(rows, colss)],
                       gy[np.ix_(rows, colss)]], axis=1)
        in_maps.append({
            "zs": np.ascontiguousarray(zs),
            "ms": np.ascontiguousarray(ms[r0: r0 + ROWS, :]),
            "lhs": lhs,
            "lhsb": lhsb,
            "sa": np.ascontiguousarray(sa),
            "sb": np.ascontiguousarray(sb),
        })
    return in_maps


def _install_profile_hook():
    """The image's antenv lacks axon_hooks; recreate it so trace=True can
    drive NTFF profiling through libaxon_pjrt (local-only, no upload)."""
    import sys
    import types

    if "antenv.axon_hooks" not in sys.modules:
        mod = types.ModuleType("antenv.axon_hooks")
        holder = {}
        mod.set_axon_ntff_profile_hook = lambda h: holder.__setitem__("h", h)
        mod.get_axon_ntff_profile_hook = lambda: holder.get("h")
        sys.modules["antenv.axon_hooks"] = mod
        try:
            import antenv
            antenv.axon_hooks = mod
        except ImportError:
            pass
        if "/root/.axon_site" not in sys.path:
            sys.path.insert(0, "/root/.axon_site")
        from trn_agent_boot.trn_boot import _ntff_profile_via_ctypes
        h = _ntff_profile_via_ctypes("/opt/axon/libaxon_pjrt.so")
        if h is not None:
            mod.set_axon_ntff_profile_hook(h)
    from concourse import bass_utils as bu
    bu.upload_artifacts = lambda tmpdir: ""


def kernel(grid_x, grid_y, mask, d=1, kn=100, **_unused):
    global LAST_RESULTS
    from concourse.bass_utils import run_bass_kernel_spmd
    from concourse.bass_utils import checkenv

    if checkenv("KERNEL_TRACE"):
        _install_profile_hook()

    d_val = float(np.asarray(d))
    kn_val = float(np.asarray(kn))
    key = (d_val, kn_val)
    if key not in _cache:
        _cache[key] = _build(d_val, kn_val)
    nc = _cache[key]

    gx = (np.asarray(grid_x, dtype=np.float32)[0, 0]
          * np.float32(SCALE)).astype(np.float16)
    gy = (np.asarray(grid_y, dtype=np.float32)[0, 0]
          * np.float32(SCALE)).astype(np.float16)
    ms = np.asarray(mask, dtype=np.float32)[0, 0].astype(np.float16)
    in_maps = _host_inputs(gx, gy, ms)

    res = run_bass_kernel_spmd(nc, in_maps, core_ids=list(range(NCORES)),
                               trace=bool(checkenv("KERNEL_TRACE")))
    LAST_RESULTS = res

    fx = np.concatenate([res.results[i]["out"][0] for i in range(NCORES)],
                        axis=0)
    fy = np.concatenate([res.results[i]["out"][1] for i in range(NCORES)],
                        axis=0)
    fx = fx.reshape(1, 1, NY, NX)
    fy = fy.reshape(1, 1, NY, NX)
    return fx, fy


# revision 19
# speedup vs baseline: 1.0202x; 1.0202x over previous
"""AI4DEM contact-force stencil on 8 TRN2 NeuronCores.

Math: for each neighbor offset o=(oy,ox) in the 5x5 window,
  dx = x - shift(x, o), dy likewise, dist = sqrt(dx^2+dy^2)
  Fx_o = where(dist < 2d, kn*(dist-2d)/max(eps,dist) * dx, 0)
       = -kn * relu(2d/dist - 1) * dx    (clamped at dist<eps)
  fx = mask * sum_o Fx_o

Mirror symmetry: Fx_{-o}(p) = -Fx_o(p+o), so only the 12 half-offsets
H = {oy>0} u {oy=0, ox>0} are computed:
  fx(p) = sum_{o in H} px_o(p) - px_o(p+o),   px_o = w_o * dx_o
The shifted term is accumulated by TensorE matmuls with +/-1 shift
matrices into PSUM (partition shifts), free-dim reads handle ox.
The 2 rows past each 128-row tile (and past the core's 256-row block)
come from one packed 48-partition "seam" chain over host-gathered rows
(each seam row pre-shifted by its own ox so all 5 ox-blocks merge into
a single lhsT per tile => 16 seam matmuls instead of 80).

Engine split per offset-chain (the elementwise bottleneck):
 - x and y slabs are packed side by side in ONE fp16 tensor, so dx and
   dy come from a single paired DVE subtract in 2x mode.
 - squares: one paired ACT Square (ACT is dtype-insensitive).
 - sq add + the contact weight on DVE; the weight uses tensor_scalar
   (max c, sub c) in 4x mode: relu(a*r-1) = a*(max(r,1/a)-1/a), with a
   folded into the final output scale.
 - px multiply on DVE (fp16 2x), py multiply on the otherwise-idle
   GpSimd, output scale+mask stt on GpSimd.

1/dist is one ACT op Abs_reciprocal_sqrt(sq + 4.1e-6); grids are
pre-scaled by 64 then cast to fp16 on the host so fp16 sq stays normal
for all dist >= ~eps; all scale factors fold into existing constants.
"""

import numpy as np

NY = NX = 2048
NCORES = 8
ROWS = NY // NCORES          # 256 rows per core
TILE = 128
NT = ROWS // TILE            # 2 row-tiles per core
W = NX + 4                   # px width (2-col halo)
WS = NX + 8                  # slab width (4-col halo)
EPS = 1e-4
SCALE = 64.0

# half-offsets, oy ascending so oy=0 pairs start right after the first loads
HOFF = [(0, 1), (0, 2)] + [(oy, ox) for oy in (1, 2) for ox in (-2, -1, 0, 1, 2)]
NH = len(HOFF)               # 12

_cache = {}
LAST_RESULTS = None


def _build(d_val: float, kn_val: float):
    import concourse.tile as tile
    from concourse import bacc, mybir

    f32 = mybir.dt.float32
    f16 = mybir.dt.float16
    AF = mybir.ActivationFunctionType
    OP = mybir.AluOpType

    nc = bacc.Bacc("TRN2", target_bir_lowering=False, debug=False,
                   enable_asserts=False, num_devices=NCORES)
    # packed x|y slabs, fp16: [row, half(x=0,y=1), col]
    zs_ext = nc.declare_dram_parameter("zs", [ROWS + 4, 2, WS], f16,
                                       isOutput=False)
    ms_ext = nc.declare_dram_parameter("ms", [ROWS, NX], f16, isOutput=False)
    # lhs blocks: [I, -S0, -S1, -S2, _, I-S1, I-S2] where Sk shifts partitions
    lhs_ext = nc.declare_dram_parameter("lhs", [128, 6 * 128], f16,
                                        isOutput=False)
    # merged boundary lhsT per t: [48, 2*128]
    lhsb_ext = nc.declare_dram_parameter("lhsb", [48, 2 * 128], f16,
                                         isOutput=False)
    # seam a/b rows, packed x|y, per-row pre-shifted by ox; each 2052-wide
    # row split across two partitions (compact form halves seam op cost)
    sa_ext = nc.declare_dram_parameter("sa", [8 * NH, 2, 1026], f16,
                                       isOutput=False)
    sb_ext = nc.declare_dram_parameter("sb", [8 * NH, 2, 1026], f16,
                                       isOutput=False)
    out_ext = nc.declare_dram_parameter("out", [2, ROWS, NX], f32,
                                        isOutput=True)

    SP = 4 * NH              # 48 seam partitions
    cw = 1.0 / (2.0 * SCALE * d_val)     # 1/a for the weight tensor_scalar

    with tile.TileContext(nc) as tc:
        with tc.tile_pool(name="const", bufs=1) as cpool, \
             tc.tile_pool(name="zin", bufs=2) as zpool, \
             tc.tile_pool(name="deep", bufs=4) as dpool, \
             tc.tile_pool(name="sqp", bufs=2) as qpool, \
             tc.tile_pool(name="tmp", bufs=2) as tpool, \
             tc.tile_pool(name="pxy", bufs=3) as ppool, \
             tc.tile_pool(name="outp", bufs=2) as opool, \
             tc.tile_pool(name="acc", bufs=1, space="PSUM") as psum_pool:

            lhs_t = cpool.tile([128, 6 * 128], f16)
            nc.sync.dma_start(lhs_t[:], lhs_ext[:])
            lhsb_t = cpool.tile([48, 2 * 128], f16)
            nc.sync.dma_start(lhsb_t[:], lhsb_ext[:])
            floor_b = cpool.tile([128, 1], f32)
            nc.vector.memset(floor_b[:], 4.1e-6)
            negc_b = cpool.tile([128, 1], f32)
            nc.vector.memset(negc_b[:], -cw)
            pxm = cpool.tile([SP, W], f16, tag="pxm")
            pym = cpool.tile([SP, W], f16, tag="pym")

            oscale = -2.0 * float(kn_val) * float(d_val)
            NCH = NT * NH                    # 24 interior chains
            # per-tile processing order: ox==0 offsets last (cheap tail: the
            # merged I-S pass needs half the matmuls of an ox!=0 offset)
            POFF = [0, 1, 2, 3, 5, 6, 7, 8, 10, 11, 4, 9]
            chains = [(t, oi) for t in range(NT) for oi in POFF]
            # w via ACT Relu on these chains (engine balance), else DVE ts
            w_on_act = {g for g, (t, oi) in enumerate(chains)
                        if oi % 2 == 0 and oi != 6}

            Z = {}       # Z[t][s] slab tiles
            MS = {}      # mask per tile
            FX = {}      # psum chunk tiles per tile
            FY = {}
            st = {}      # per-chain tile state
            sst = {}     # seam state

            def emit_zdma(t):
                t0 = t * TILE
                Z[t] = {}
                for s in (0, -1, -2):
                    zt = zpool.tile([TILE, 2, WS], f16, tag=f"zs{s}",
                                    name=f"z{t}{s}")
                    nc.sync.dma_start(
                        zt[:], zs_ext[t0 + s + 2: t0 + s + 2 + TILE, :, :])
                    Z[t][s] = zt
                mt = zpool.tile([TILE, NX], f16, tag="mask", name=f"m{t}")
                nc.sync.dma_start(mt[:], ms_ext[t0: t0 + TILE, :])
                MS[t] = mt

            def emit_sub(g):
                t, oi = chains[g]
                oy, ox = HOFF[oi]
                dxy = dpool.tile([TILE, 2, W], f16, tag="dxy", name=f"dxy{g}")
                nc.vector.tensor_sub(
                    dxy[:], Z[t][0][:, :, 2: 2 + W],
                    Z[t][-oy][:, :, 2 - ox: 2 - ox + W])
                st[g] = {"dxy": dxy}

            def emit_sq(g):
                sq2 = qpool.tile([TILE, 2, W], f16, tag="sq2", name=f"sq2{g}")
                nc.scalar.activation(sq2[:], st[g]["dxy"][:], AF.Square)
                st[g]["sq2"] = sq2

            def emit_add(g):
                sq2 = st[g].pop("sq2")
                rec = tpool.tile([TILE, W], f16, tag="rec", name=f"rec{g}")
                nc.vector.tensor_add(rec[:], sq2[:, 0, :], sq2[:, 1, :])
                st[g]["rec"] = rec

            def emit_rsqrt(g):
                rec = st[g]["rec"]
                nc.scalar.activation(rec[:], rec[:], AF.Abs_reciprocal_sqrt,
                                     bias=floor_b[:TILE])
                if g in w_on_act:
                    w = tpool.tile([TILE, W], f16, tag="w", name=f"w{g}")
                    nc.scalar.activation(w[:], rec[:], AF.Relu,
                                         bias=negc_b[:TILE])
                    st[g]["w"] = w

            def emit_tail(g):
                t, oi = chains[g]
                oy, ox = HOFF[oi]
                first = g % NH == 0
                last = g % NH == NH - 1
                fin = g == NCH - 1      # last chain of the whole kernel
                if g not in w_on_act:
                    w = tpool.tile([TILE, W], f16, tag="w", name=f"w{g}")
                    nc.vector.tensor_scalar(w[:], st[g]["rec"][:], cw, cw,
                                            OP.max, OP.subtract)
                    st[g]["w"] = w
                w = st[g]["w"]
                dxy = st[g]["dxy"]
                px = ppool.tile([TILE, W], f16, tag="px", name=f"px{g}")
                py = ppool.tile([TILE, W], f16, tag="py", name=f"py{g}")
                nc.vector.tensor_mul(px[:], w[:], dxy[:, 0, :])
                nc.vector.tensor_mul(py[:], w[:], dxy[:, 1, :])
                if first:
                    FX[t] = [psum_pool.tile([TILE, 512], f32, tag=f"fx{k}",
                                            name=f"fx{t}{k}")
                             for k in range(NX // 512)]
                    FY[t] = [psum_pool.tile([TILE, 512], f32, tag=f"fy{k}",
                                            name=f"fy{t}{k}")
                             for k in range(NX // 512)]
                for ps, acc, drain_ks in ((px, FX[t], [0, 1, 2, 3]),
                                          (py, FY[t], None)):
                    if ox == 0:
                        for k in range(NX // 512):
                            nc.tensor.matmul(
                                acc[k][:],
                                lhs_t[:, 128 * (3 + oy): 128 * (4 + oy)],
                                ps[:, 2 + 512 * k: 2 + 512 * k + 512],
                                start=first, stop=last)
                            if fin:
                                # drain each chunk right after its stop mm
                                emit_drain_one(t, k, FX[t] if drain_ks
                                               else FY[t],
                                               0 if drain_ks else 1)
                    else:
                        for k in range(NX // 512):
                            nc.tensor.matmul(
                                acc[k][:], lhs_t[:, 0:128],
                                ps[:, 2 + 512 * k: 2 + 512 * k + 512],
                                start=first, stop=False)
                        for k in range(NX // 512):
                            nc.tensor.matmul(
                                acc[k][:],
                                lhs_t[:, 128 * (1 + oy): 128 * (2 + oy)],
                                ps[:, 2 + 512 * k + ox:
                                   2 + 512 * k + ox + 512],
                                start=False, stop=last)
                            if fin:
                                emit_drain_one(t, k, FX[t] if drain_ks
                                               else FY[t],
                                               0 if drain_ks else 1)
                del st[g]

            def emit_seam_mms(t):
                for pm, acc in ((pxm, FX[t]), (pym, FY[t])):
                    for k in range(NX // 512):
                        nc.tensor.matmul(
                            acc[k][:], lhsb_t[:, 128 * t: 128 * t + 128],
                            pm[:, 2 + 512 * k: 2 + 512 * k + 512],
                            start=False, stop=False)

            def emit_drain_one(t, k, acc, half, nsplit=1):
                t0 = t * TILE
                tagc = "fxsb" if half == 0 else "fysb"
                sb_t = opool.tile([TILE, 512], f32, tag=tagc,
                                  name=f"{tagc}{t}{k}")
                nc.vector.scalar_tensor_tensor(
                    sb_t[:], acc[k][:], oscale,
                    MS[t][:, 512 * k: 512 * k + 512], OP.mult, OP.mult)
                sw = 512 // nsplit
                for j in range(nsplit):
                    c0 = 512 * k + sw * j
                    nc.sync.dma_start(
                        out_ext[half, t0: t0 + TILE, c0: c0 + sw],
                        sb_t[:, sw * j: sw * j + sw])

            def emit_drain(t, ks):
                for k in ks:
                    emit_drain_one(t, k, FX[t], 0)
                    emit_drain_one(t, k, FY[t], 1)

            SPC = 2 * SP        # 96 compact seam partitions, 1026 wide

            def emit_seam_stage(stage):
                if stage == 0:
                    sst["sa"] = zpool.tile([SPC, 2, 1026], f16, tag="sa",
                                           bufs=1, name="sa")
                    nc.sync.dma_start(sst["sa"][:], sa_ext[:])
                    sst["sb"] = zpool.tile([SPC, 2, 1026], f16, tag="sb",
                                           bufs=1, name="sb")
                    nc.sync.dma_start(sst["sb"][:], sb_ext[:])
                elif stage == 1:
                    sst["dm"] = cpool.tile([SPC, 2, 1026], f16, tag="sdm",
                                           name="sdm")
                    nc.vector.tensor_sub(sst["dm"][:], sst["sa"][:],
                                         sst["sb"][:])
                elif stage == 2:
                    sst["s2m"] = cpool.tile([SPC, 2, 1026], f16, tag="ss2",
                                            name="ss2")
                    nc.scalar.activation(sst["s2m"][:], sst["dm"][:],
                                         AF.Square)
                elif stage == 3:
                    s2m = sst["s2m"]
                    sst["recm"] = cpool.tile([SPC, 1026], f16, tag="srec",
                                             name="srec")
                    nc.vector.tensor_add(sst["recm"][:], s2m[:, 0, :],
                                         s2m[:, 1, :])
                elif stage == 4:
                    nc.scalar.activation(sst["recm"][:], sst["recm"][:],
                                         AF.Abs_reciprocal_sqrt,
                                         bias=floor_b[:SPC])
                elif stage == 5:
                    sst["wm"] = cpool.tile([SPC, 1026], f16, tag="swm",
                                           name="swm")
                    nc.scalar.activation(sst["wm"][:], sst["recm"][:],
                                         AF.Relu, bias=negc_b[:SPC])
                elif stage == 6:
                    sst["pxc"] = cpool.tile([SPC, 1026], f16, tag="spxc",
                                            name="spxc")
                    nc.vector.tensor_mul(sst["pxc"][:], sst["wm"][:],
                                         sst["dm"][:, 0, :])
                    for j in range(2):
                        nc.sync.dma_start(
                            pxm[:, 1026 * j: 1026 * j + 1026],
                            sst["pxc"][j::2, :])
                elif stage == 7:
                    sst["pyc"] = cpool.tile([SPC, 1026], f16, tag="spyc",
                                            name="spyc")
                    nc.vector.tensor_mul(sst["pyc"][:], sst["wm"][:],
                                         sst["dm"][:, 1, :])
                    for j in range(2):
                        nc.sync.dma_start(
                            pym[:, 1026 * j: 1026 * j + 1026],
                            sst["pyc"][j::2, :])

            # ---- stage-skewed software pipeline over the 24 chains
            emit_zdma(0)
            for e in range(NCH + 2):
                if e == 0:
                    emit_sub(0)
                    emit_sub(1)
                    emit_sq(0)
                if e < NCH:
                    emit_add(e)
                if e + 1 < NCH:
                    emit_sq(e + 1)
                if e + 2 < NCH:
                    emit_sub(e + 2)
                if e < NCH:
                    emit_rsqrt(e)
                if 1 <= e < 9:
                    emit_seam_stage(e - 1)
                if e == 6:
                    emit_zdma(1)
                g = e - 1
                if 0 <= g < NCH:
                    emit_tail(g)
                    if g == 8:
                        emit_seam_mms(0)
                    if g == NH + 1:
                        emit_seam_mms(1)
                    if g == NH - 1:
                        emit_drain(0, [0, 1])
                    if g == NH:
                        emit_drain(0, [2, 3])

    nc.compile()
    return nc


def _host_inputs(gx, gy, ms):
    """Per-core input dict list. gx/gy scaled by 64 and cast to fp16."""
    eye = np.eye(128, dtype=np.float16)
    lhs = np.zeros((128, 6 * 128), dtype=np.float16)
    lhs[:, 0:128] = eye
    for oy in (0, 1, 2):
        blk = np.zeros((128, 128), dtype=np.float16)
        for m in range(128 - oy):
            blk[m + oy, m] = -1.0
        lhs[:, 128 * (1 + oy): 128 * (2 + oy)] = blk
        if oy > 0:
            lhs[:, 128 * (3 + oy): 128 * (4 + oy)] = eye + blk
    # merged boundary lhsT: rows are disjoint per offset, so sum over ox
    lhsb = np.zeros((48, 2 * 128), dtype=np.float16)
    for t in (0, 1):
        blk = np.zeros((48, 128), dtype=np.float16)
        for oi, (oy, ox) in enumerate(HOFF):
            if oy == 0:
                continue
            for m in range(128 - oy, 128):
                blk[4 * oi + 2 * t + (m + oy - 128), m] = -1.0
        lhsb[:, 128 * t: 128 * t + 128] = blk

    cols = np.arange(-2, NX + 2) % NX         # width W, col u -> u-2
    colss = np.arange(-4, NX + 4) % NX        # width WS, col v -> v-4
    in_maps = []
    for i in range(NCORES):
        r0 = i * ROWS
        rows = np.arange(r0 - 2, r0 + ROWS + 2) % NY
        # seam rows per (oi, j): j in {0,1}: r0+128+j ; j in {2,3}: r0+256+(j-2)
        # each row pre-shifted by its own ox: sa col c -> u = c-2+ox
        sa_rows = np.empty(4 * NH, dtype=np.int64)
        sb_rows = np.empty(4 * NH, dtype=np.int64)
        sa_cols = np.empty((4 * NH, W), dtype=np.int64)
        for oi, (oy, ox) in enumerate(HOFF):
            for j in range(4):
                row = r0 + 128 + j if j < 2 else r0 + 256 + (j - 2)
                sa_rows[4 * oi + j] = row % NY
                sb_rows[4 * oi + j] = (row - oy) % NY
                sa_cols[4 * oi + j] = (cols + ox) % NX
        sa = np.stack([gx[sa_rows[:, None], sa_cols],
                       gy[sa_rows[:, None], sa_cols]], axis=1)
        sb = np.stack([gx[sb_rows][:, cols],
                       gy[sb_rows][:, cols]], axis=1)
        # compact seam form: row r col c -> partition 2r+c//1026, col c%1026
        sa = sa.reshape(4 * NH, 2, 2, 1026).transpose(0, 2, 1, 3)
        sa = sa.reshape(8 * NH, 2, 1026)
        sb = sb.reshape(4 * NH, 2, 2, 1026).transpose(0, 2, 1, 3)
        sb = sb.reshape(8 * NH, 2, 1026)
        zs = np.stack([gx[np.ix_(rows, colss)],
                       gy[np.ix_(rows, colss)]], axis=1)
        in_maps.append({
            "zs": np.ascontiguousarray(zs),
            "ms": np.ascontiguousarray(ms[r0: r0 + ROWS, :]),
            "lhs": lhs,
            "lhsb": lhsb,
            "sa": np.ascontiguousarray(sa),
            "sb": np.ascontiguousarray(sb),
        })
    return in_maps


def _install_profile_hook():
    """The image's antenv lacks axon_hooks; recreate it so trace=True can
    drive NTFF profiling through libaxon_pjrt (local-only, no upload)."""
    import sys
    import types

    if "antenv.axon_hooks" not in sys.modules:
        mod = types.ModuleType("antenv.axon_hooks")
        holder = {}
        mod.set_axon_ntff_profile_hook = lambda h: holder.__setitem__("h", h)
        mod.get_axon_ntff_profile_hook = lambda: holder.get("h")
        sys.modules["antenv.axon_hooks"] = mod
        try:
            import antenv
            antenv.axon_hooks = mod
        except ImportError:
            pass
        if "/root/.axon_site" not in sys.path:
            sys.path.insert(0, "/root/.axon_site")
        from trn_agent_boot.trn_boot import _ntff_profile_via_ctypes
        h = _ntff_profile_via_ctypes("/opt/axon/libaxon_pjrt.so")
        if h is not None:
            mod.set_axon_ntff_profile_hook(h)
    from concourse import bass_utils as bu
    bu.upload_artifacts = lambda tmpdir: ""


def kernel(grid_x, grid_y, mask, d=1, kn=100, **_unused):
    global LAST_RESULTS
    from concourse.bass_utils import run_bass_kernel_spmd
    from concourse.bass_utils import checkenv

    if checkenv("KERNEL_TRACE"):
        _install_profile_hook()

    d_val = float(np.asarray(d))
    kn_val = float(np.asarray(kn))
    key = (d_val, kn_val)
    if key not in _cache:
        _cache[key] = _build(d_val, kn_val)
    nc = _cache[key]

    gx = (np.asarray(grid_x, dtype=np.float32)[0, 0]
          * np.float32(SCALE)).astype(np.float16)
    gy = (np.asarray(grid_y, dtype=np.float32)[0, 0]
          * np.float32(SCALE)).astype(np.float16)
    ms = np.asarray(mask, dtype=np.float32)[0, 0].astype(np.float16)
    in_maps = _host_inputs(gx, gy, ms)

    res = run_bass_kernel_spmd(nc, in_maps, core_ids=list(range(NCORES)),
                               trace=bool(checkenv("KERNEL_TRACE")))
    LAST_RESULTS = res

    fx = np.concatenate([res.results[i]["out"][0] for i in range(NCORES)],
                        axis=0)
    fy = np.concatenate([res.results[i]["out"][1] for i in range(NCORES)],
                        axis=0)
    fx = fx.reshape(1, 1, NY, NX)
    fy = fy.reshape(1, 1, NY, NX)
    return fx, fy
